# revision 1
# baseline (speedup 1.0000x reference)
"""GAT 2-layer kernel for Trainium2 (8 NeuronCores, Bass/Tile).

Strategy:
 - Nodes partitioned contiguously across 8 cores by dst; edges assigned to the
   core owning their dst (incl. self-loops).
 - Layer-1 attention logits (a_src.x / a_dst.x per node) are linear in x, so
   alpha1 is precomputed on host; layer-1 aggregates x rows (128-wide) and
   applies W1 *after* aggregation (h = x@W1 is linear).
 - Edge gathers use the gpsimd dma_gather ucode (int16 indices -> tables are
   split in 4 quarters; 3 SWDGE queues round-robin).
 - Scatter-add is done on the TensorEngine: per 128-edge chunk a one-hot
   (slot==iota) mask scaled by the edge weight forms the rhs of a matmul that
   accumulates into a per-4-block PSUM quad.
 - Layer-2 attention is computed on device: asrc2/adst2 ride the AllGathered
   h2 table rows; exp(leaky(z)) = max(exp(z), exp(0.2 z)).
 - One AllGather (bf16) exchanges h2 + attention scalars between layers.
"""

import os
import sys

for p in ("/opt/trn_rl_repo", "/root/.axon_site/_ro/trn_rl_repo"):
    if os.path.isdir(p) and p not in sys.path:
        sys.path.insert(0, p)

import math

import ml_dtypes
import numpy as np

import concourse.bacc as bacc
import concourse.bass as bass
import concourse.mybir as mybir
import concourse.tile as tile
from concourse.bass_utils import run_bass_kernel_spmd

F32 = mybir.dt.float32
BF16 = mybir.dt.bfloat16
I16 = mybir.dt.int16
AF = mybir.ActivationFunctionType
OP = mybir.AluOpType
BF = ml_dtypes.bfloat16

NC = 8
P = 128
NEG_SLOPE = 0.2
SGB = 4          # blocks per supergroup
NQ = 4           # table quarters
ROW2 = 384       # bf16 cols per h2-table row (768B): h2[256] | asrc2 f32 | adst2 f32 | pad
KILL_SLOT = 200.0


class Cfg:
    def __init__(self, n, e, f1, f2, f3):
        assert n % (NC * 4) == 0
        self.N, self.E, self.F1, self.F2, self.F3 = n, e, f1, f2, f3
        self.NLOC = n // NC                      # owned dst nodes per core
        self.NB = (self.NLOC + P - 1) // P       # 128-dst blocks per core
        self.NPAD = self.NB * P
        self.NSG = (self.NB + SGB - 1) // SGB
        self.Q1R = n // NQ                       # rows per x-table quarter
        self.NTOT = NC * self.NPAD               # AllGather table rows
        assert self.NTOT % NQ == 0
        self.Q2R = self.NTOT // NQ
        assert self.Q1R < 32768 and self.Q2R < 32768


def _leaky(v):
    return np.where(v > 0, v, NEG_SLOPE * v)


def _pack_idx(per_inst_idx, total_cols):
    """Pack per-instruction index lists into the [128, cols] int16 SBUF layout:
    idx j of instruction i -> partition j%16 (replicated to all 8 groups of
    16), column col0_i + j//16."""
    arr = np.zeros((P, total_cols), dtype=np.int16)
    col0 = 0
    for ids in per_inst_idx:
        ni = len(ids)
        if ni == 0:
            continue
        w = np.asarray(ids, dtype=np.int16).reshape(ni // 16, 16).T  # [16, ni/16]
        arr[:, col0:col0 + ni // 16] = np.tile(w, (8, 1))
        col0 += ni // 16
    return arr


def preprocess(cfg, inputs):
    """Host-side graph partitioning + layer-1 attention + per-core data."""
    x = np.asarray(inputs["x"], dtype=np.float32)
    ei = np.asarray(inputs["edge_index"])
    W1 = np.asarray(inputs["W1"], np.float32)
    a1s = np.asarray(inputs["a1_src"], np.float32)
    a1d = np.asarray(inputs["a1_dst"], np.float32)
    W2 = np.asarray(inputs["W2"], np.float32)
    a2s = np.asarray(inputs["a2_src"], np.float32)
    a2d = np.asarray(inputs["a2_dst"], np.float32)

    n = cfg.N
    loop = np.arange(n, dtype=np.int64)
    src = np.concatenate([ei[0], loop]).astype(np.int64)
    dst = np.concatenate([ei[1], loop]).astype(np.int64)

    # layer-1 attention entirely from host matvecs (linear in x)
    asrc1 = x @ (W1 @ a1s)
    adst1 = x @ (W1 @ a1d)
    w1e = np.exp(_leaky(asrc1[src] + adst1[dst])).astype(np.float32)
    den1 = np.bincount(dst, weights=w1e, minlength=n).astype(np.float32)
    alpha1 = (w1e / den1[dst]).astype(np.float32)

    core = dst // cfg.NLOC
    dloc = dst - core * cfg.NLOC
    blk = dloc // P
    slot = (dloc % P).astype(np.float32)
    q1 = src // cfg.Q1R
    i1 = (src - q1 * cfg.Q1R).astype(np.int32)
    ksrc = src // cfg.NLOC
    grow = ksrc * cfg.NPAD + (src - ksrc * cfg.NLOC)
    q2 = grow // cfg.Q2R
    i2 = (grow - q2 * cfg.Q2R).astype(np.int32)

    # deterministic shuffle key to avoid monotone index runs inside groups
    hkey = (src * 2654435761) % (1 << 31)

    # per-core, per-(block, quarter) edge lists for both layers
    groups1 = [[[None] * NQ for _ in range(cfg.NB)] for _ in range(NC)]
    groups2 = [[[None] * NQ for _ in range(cfg.NB)] for _ in range(NC)]
    for k in range(NC):
        m = core == k
        for (qv, groups) in ((q1[m], groups1[k]), (q2[m], groups2[k])):
            order = np.lexsort((hkey[m], qv, blk[m]))
            bq = (blk[m][order] * NQ + qv[order]).astype(np.int64)
            cnt = np.bincount(bq, minlength=cfg.NB * NQ)
            off = np.concatenate([[0], np.cumsum(cnt)])
            for b in range(cfg.NB):
                for q in range(NQ):
                    g = bq_idx = b * NQ + q
                    sel = order[off[g]:off[g + 1]]
                    groups[b][q] = sel  # indices into the core-masked arrays
        # stash masked per-core arrays
    percore = []
    for k in range(NC):
        m = core == k
        percore.append(dict(slot=slot[m], alpha=alpha1[m], i1=i1[m], i2=i2[m]))

    # shared chunk structure: chunks per (b, q) = max over cores
    nch1 = np.zeros((cfg.NB, NQ), np.int64)
    nch2 = np.zeros((cfg.NB, NQ), np.int64)
    for k in range(NC):
        for b in range(cfg.NB):
            for q in range(NQ):
                nch1[b, q] = max(nch1[b, q], -(-len(groups1[k][b][q]) // P))
                nch2[b, q] = max(nch2[b, q], -(-len(groups2[k][b][q]) // P))
    # every block needs >=1 chunk total (self loops guarantee edges exist)

    def build_layer(nch, groups_all, idx_key, with_alpha):
        # static metadata (same all cores)
        chunks = []          # (sg, q, b, first, last)
        insts = []           # (sg, q, nchunks, col0, chunk0)
        col0 = 0
        cglob = 0
        # first/last chunk per block across q order
        firsts, lasts = {}, {}
        for sg in range(cfg.NSG):
            blocks = range(sg * SGB, min((sg + 1) * SGB, cfg.NB))
            for q in range(NQ):
                nc_inst = int(sum(nch[b, q] for b in blocks))
                insts.append((sg, q, nc_inst, col0, cglob))
                for b in blocks:
                    for j in range(int(nch[b, q])):
                        chunks.append([sg, q, b, False, False])
                        if b not in firsts:
                            firsts[b] = cglob
                        lasts[b] = cglob
                        cglob += 1
                col0 += nc_inst * P // 16
        for b, c in firsts.items():
            chunks[c][3] = True
        for b, c in lasts.items():
            chunks[c][4] = True
        C = cglob

        # per-core data arrays
        data = []
        for k in range(NC):
            pc = percore[k]
            sl = np.full((P, C), KILL_SLOT, np.float32)
            al = np.zeros((P, C), np.float32) if with_alpha else None
            rng = np.random.default_rng(12345 + k)
            per_inst_idx = []
            ci = 0
            for (sg, q, nc_inst, c0, ch0) in insts:
                ids = []
                for b in range(sg * SGB, min((sg + 1) * SGB, cfg.NB)):
                    sel = groups_all[k][b][q]
                    nchunks = int(nch[b, q])
                    npad = nchunks * P
                    iv = pc[idx_key][sel]
                    pad = npad - len(sel)
                    if pad:
                        iv = np.concatenate([iv, rng.integers(0, 997, size=pad)])
                    ids.append(iv)
                    svec = np.full(npad, KILL_SLOT, np.float32)
                    svec[:len(sel)] = pc["slot"][sel]
                    sl[:, ci:ci + nchunks] = svec.reshape(nchunks, P).T
                    if with_alpha:
                        avec = np.zeros(npad, np.float32)
                        avec[:len(sel)] = pc["alpha"][sel]
                        al[:, ci:ci + nchunks] = avec.reshape(nchunks, P).T
                    ci += nchunks
                per_inst_idx.append(np.concatenate(ids) if ids else np.zeros(0, np.int64))
            idxarr = _pack_idx(per_inst_idx, col0)
            d = dict(idx=idxarr, sl=sl.astype(BF))
            if with_alpha:
                d["al"] = al.astype(np.float32)
            data.append(d)
        return dict(chunks=chunks, insts=insts, C=C, cols=col0, data=data)

    L1 = build_layer(nch1, groups1, "i1", True)
    L2 = build_layer(nch2, groups2, "i2", False)

    # shared small tensors
    iota = np.tile(np.arange(P, dtype=np.float32)[None, :], (P, 1)).astype(BF)
    shared = dict(
        xq=x.astype(BF),
        w1f=W1.astype(np.float32),
        w2f=W2.astype(np.float32),
        b1t=np.tile(np.asarray(inputs["b1"], np.float32)[None, :], (P, 1)),
        b2t=np.tile(np.asarray(inputs["b2"], np.float32)[None, :], (P, 1)),
        a2st=np.tile((W2 @ a2s).astype(np.float32)[None, :], (P, 1)).astype(BF),
        a2dt=np.tile((W2 @ a2d).astype(np.float32)[None, :], (P, 1)).astype(BF),
        iot=iota,
    )
    in_maps = []
    for k in range(NC):
        m = dict(shared)
        m["idx1"] = L1["data"][k]["idx"]
        m["sl1"] = L1["data"][k]["sl"]
        m["al1"] = L1["data"][k]["al"]
        m["idx2"] = L2["data"][k]["idx"]
        m["sl2"] = L2["data"][k]["sl"]
        in_maps.append(m)
    meta = dict(L1=L1, L2=L2)
    return meta, in_maps


def build(cfg, meta, phase="all"):
    nc = bacc.Bacc("TRN2", target_bir_lowering=False, debug=False,
                   num_devices=NC, num_swdge_queues=3)
    L1, L2 = meta["L1"], meta["L2"]
    F1, F2, F3 = cfg.F1, cfg.F2, cfg.F3

    xq = nc.dram_tensor("xq", [cfg.N, F1], BF16, kind="ExternalInput")
    w1f = nc.dram_tensor("w1f", [F1, F2], F32, kind="ExternalInput")
    w2f = nc.dram_tensor("w2f", [F2, F3], F32, kind="ExternalInput")
    b1t = nc.dram_tensor("b1t", [P, F2], F32, kind="ExternalInput")
    b2t = nc.dram_tensor("b2t", [P, F3], F32, kind="ExternalInput")
    a2st = nc.dram_tensor("a2st", [P, F2], BF16, kind="ExternalInput")
    a2dt = nc.dram_tensor("a2dt", [P, F2], BF16, kind="ExternalInput")
    iot_d = nc.dram_tensor("iot", [P, P], BF16, kind="ExternalInput")
    idx1 = nc.dram_tensor("idx1", [P, L1["cols"]], I16, kind="ExternalInput")
    sl1 = nc.dram_tensor("sl1", [P, L1["C"]], BF16, kind="ExternalInput")
    al1 = nc.dram_tensor("al1", [P, L1["C"]], F32, kind="ExternalInput")
    idx2 = nc.dram_tensor("idx2", [P, L2["cols"]], I16, kind="ExternalInput")
    sl2 = nc.dram_tensor("sl2", [P, L2["C"]], BF16, kind="ExternalInput")
    outy = nc.dram_tensor("outy", [cfg.NPAD, F3], F32, kind="ExternalOutput")

    with tile.TileContext(nc) as tc:
        with tc.tile_pool(name="dram", bufs=1, space="DRAM") as dpool, \
             tc.tile_pool(name="const", bufs=1) as cp:
            h2tab = dpool.tile([cfg.NPAD, ROW2], BF16)
            agout = dpool.tile([cfg.NTOT, ROW2], BF16)
            adsto = dpool.tile([cfg.NB, P], F32)

            w1s = cp.tile([F1, F2], F32)
            nc.sync.dma_start(out=w1s[:], in_=w1f[:, :])
            w2as = cp.tile([P, F3], F32)
            nc.sync.dma_start(out=w2as[:], in_=w2f[0:P, :])
            w2bs = cp.tile([P, F3], F32)
            nc.sync.dma_start(out=w2bs[:], in_=w2f[P:2 * P, :])
            b1s = cp.tile([P, F2], F32)
            nc.sync.dma_start(out=b1s[:], in_=b1t[:, :])
            b2s = cp.tile([P, F3], F32)
            nc.sync.dma_start(out=b2s[:], in_=b2t[:, :])
            a2ss = cp.tile([P, F2], BF16)
            nc.sync.dma_start(out=a2ss[:], in_=a2st[:, :])
            a2ds = cp.tile([P, F2], BF16)
            nc.sync.dma_start(out=a2ds[:], in_=a2dt[:, :])
            iot = cp.tile([P, P], BF16)
            nc.sync.dma_start(out=iot[:], in_=iot_d[:, :])
            ones_bf = cp.tile([P, 1], BF16)
            nc.vector.memset(ones_bf[:], 1.0)
            zeroL = cp.tile([8, P], BF16)
            nc.vector.memset(zeroL[:], 0.0)
            zeroR = cp.tile([8, SGB * P], BF16)
            nc.vector.memset(zeroR[:], 0.0)
            idx1s = cp.tile([P, L1["cols"]], I16)
            nc.sync.dma_start(out=idx1s[:], in_=idx1[:, :])
            sl1s = cp.tile([P, L1["C"]], BF16)
            nc.sync.dma_start(out=sl1s[:], in_=sl1[:, :])
            al1s = cp.tile([P, L1["C"]], F32)
            nc.sync.dma_start(out=al1s[:], in_=al1[:, :])
            idx2s = cp.tile([P, L2["cols"]], I16)
            nc.sync.dma_start(out=idx2s[:], in_=idx2[:, :])
            sl2s = cp.tile([P, L2["C"]], BF16)
            nc.sync.dma_start(out=sl2s[:], in_=sl2[:, :])

            qn = [0]

            def next_q():
                qn[0] = (qn[0] + 1) % 3
                return qn[0]

            # ---------------- Layer 1 ----------------
            with tc.tile_pool(name="ps1", bufs=2, space="PSUM") as pp1, \
                 tc.tile_pool(name="g1", bufs=3) as gp1, \
                 tc.tile_pool(name="m1", bufs=4) as mp1, \
                 tc.tile_pool(name="e1", bufs=2) as ep1:
                inst_by_sg = {}
                for (sg, q, nci, c0, ch0) in L1["insts"]:
                    inst_by_sg.setdefault(sg, []).append((q, nci, c0, ch0))
                sg_last1 = {}
                for c, ch in enumerate(L1["chunks"]):
                    sg_last1[ch[0]] = c
                for sg in range(cfg.NSG):
                    nblk = min(SGB, cfg.NB - sg * SGB)
                    Q = pp1.tile([P, SGB * P], F32, tag="Q1")
                    nc.tensor.matmul(out=Q[:, :], lhsT=zeroL[:], rhs=zeroR[:],
                                     start=True, stop=False)
                    for (q, nci, c0, ch0) in inst_by_sg[sg]:
                        if nci == 0:
                            continue
                        ni = nci * P
                        gt = gp1.tile([P, nci, F1], BF16, tag="g1")
                        nc.gpsimd.dma_gather(
                            gt[:, :, :], xq[q * cfg.Q1R:(q + 1) * cfg.Q1R, :],
                            idx1s[:, c0:c0 + ni // 16], ni, ni, F1,
                            single_packet=False, queue_num=next_q())
                        mask_all = mp1.tile([P, nci, P], BF16, tag="mk")
                        nc.vector.tensor_tensor(
                            out=mask_all[:],
                            in0=sl1s[:, ch0:ch0 + nci].rearrange(
                                "p (c o) -> p c o", o=1).to_broadcast([P, nci, P]),
                            in1=iot[:].rearrange("p (o f) -> p o f", o=1
                                                 ).to_broadcast([P, nci, P]),
                            op=OP.is_equal)
                        scat_all = mp1.tile([P, nci, P], BF16, tag="sc")
                        nc.vector.tensor_tensor(
                            out=scat_all[:], in0=mask_all[:],
                            in1=al1s[:, ch0:ch0 + nci].rearrange(
                                "p (c o) -> p c o", o=1).to_broadcast([P, nci, P]),
                            op=OP.mult)
                        for j in range(nci):
                            cg = ch0 + j
                            _, _, b, first, last = L1["chunks"][cg]
                            bi = b - sg * SGB
                            nc.tensor.matmul(
                                out=Q[:, bi * P:(bi + 1) * P], lhsT=gt[:, j, :],
                                rhs=scat_all[:, j, :], start=False,
                                stop=(cg == sg_last1[sg]))
                    Aq = ep1.tile([P, SGB * P], F32, tag="Aq")
                    nc.vector.tensor_copy(out=Aq[:, :nblk * P], in_=Q[:, :nblk * P])
                    for bi in range(nblk):
                        b = sg * SGB + bi
                        o1 = pp1.tile([P, F2], F32, tag="O1")
                        nc.tensor.matmul(out=o1[:], lhsT=Aq[:, bi * P:(bi + 1) * P],
                                         rhs=w1s[:], start=True, stop=True)
                        t1 = ep1.tile([P, F2], F32, tag="t1")
                        nc.vector.tensor_add(out=t1[:], in0=o1[:], in1=b1s[:])
                        h2b = ep1.tile([P, F2], BF16, tag="h2b")
                        nc.scalar.activation(out=h2b[:], in_=t1[:], func=AF.Relu)
                        import os as _os
                        _skip = _os.environ.get("GAT_SKIP", "")
                        scr = ep1.tile([P, F2], BF16, tag="scr")
                        s2a = ep1.tile([P, 1], F32, tag="s2a")
                        s2d = ep1.tile([P, 1], F32, tag="s2d")
                        if "ttr" in _skip:
                            nc.vector.memset(s2a[:], 1.0)
                            nc.vector.memset(s2d[:], 1.0)
                        else:
                            nc.vector.tensor_mul(out=scr[:], in0=h2b[:], in1=a2ss[:])
                            nc.vector.reduce_sum(s2a[:], scr[:],
                                                 mybir.AxisListType.X)
                            scr2 = ep1.tile([P, F2], BF16, tag="scr2")
                            nc.vector.tensor_mul(out=scr2[:], in0=h2b[:], in1=a2ds[:])
                            nc.vector.reduce_sum(s2d[:], scr2[:],
                                                 mybir.AxisListType.X)
                        if "h2w" not in _skip:
                            nc.sync.dma_start(out=h2tab[b * P:(b + 1) * P, 0:F2], in_=h2b[:])
                        pk = ep1.tile([P, 2], F32, tag="pk")
                        nc.vector.tensor_copy(out=pk[:, 0:1], in_=s2a[:])
                        nc.vector.tensor_copy(out=pk[:, 1:2], in_=s2d[:])
                        if "pk" not in _skip:
                            nc.sync.dma_start(
                                out=h2tab[b * P:(b + 1) * P, 2 * P:2 * P + 4].bitcast(F32),
                                in_=pk[:])
                        if "adsto" not in _skip:
                            nc.sync.dma_start(out=adsto[b:b + 1, :], in_=s2d[:])

            if phase == "l1":
                nc.sync.dma_start(out=outy[:, :],
                                  in_=h2tab[:, 0:2 * cfg.F3].bitcast(F32))
            else:
                nc.gpsimd.collective_compute(
                    "AllGather", OP.bypass,
                    replica_groups=[list(range(NC))],
                    ins=[h2tab[:].opt()], outs=[agout[:].opt()])

            # ---------------- Layer 2 ----------------
            if phase == "l1":
                pass
            else:
              with tc.tile_pool(name="ps2", bufs=2, space="PSUM") as pp2, \
                   tc.tile_pool(name="g2", bufs=3) as gp2, \
                   tc.tile_pool(name="m2", bufs=2) as mp2, \
                   tc.tile_pool(name="e2", bufs=2) as ep2, \
                   tc.tile_pool(name="ad2", bufs=2) as ap2:
                  adsr = cp.tile([cfg.NB, P], F32)
                  inst_by_sg = {}
                  for (sg, q, nci, c0, ch0) in L2["insts"]:
                      inst_by_sg.setdefault(sg, []).append((q, nci, c0, ch0))
                  sg_last2 = {}
                  for c, ch in enumerate(L2["chunks"]):
                      sg_last2[ch[0]] = c
                  for sg in range(cfg.NSG):
                      nblk = min(SGB, cfg.NB - sg * SGB)
                      Qa = pp2.tile([P, SGB * P], F32, tag="Qa")
                      Qb = pp2.tile([P, SGB * P], F32, tag="Qb")
                      den = pp2.tile([P, SGB], F32, tag="dn")
                      nc.tensor.matmul(out=Qa[:, :], lhsT=zeroL[:], rhs=zeroR[:],
                                       start=True, stop=False)
                      nc.tensor.matmul(out=Qb[:, :], lhsT=zeroL[:], rhs=zeroR[:],
                                       start=True, stop=False)
                      nc.tensor.matmul(out=den[:, :], lhsT=zeroL[:], rhs=zeroR[:, :SGB],
                                       start=True, stop=False)
                      adt = {}
                      for (q, nci, c0, ch0) in inst_by_sg[sg]:
                          if nci == 0:
                              continue
                          ni = nci * P
                          gt = gp2.tile([P, nci, ROW2], BF16, tag="g2")
                          nc.gpsimd.dma_gather(
                              gt[:, :, :], agout[q * cfg.Q2R:(q + 1) * cfg.Q2R, :],
                              idx2s[:, c0:c0 + ni // 16], ni, ni, ROW2,
                              single_packet=False, queue_num=next_q())
                          av = gt[:, :, F2:F2 + 4].bitcast(F32)[:, :, 0]
                          ea1 = mp2.tile([P, nci], F32, tag="ea1")
                          nc.scalar.activation(out=ea1[:], in_=av, func=AF.Exp)
                          ea2 = mp2.tile([P, nci], F32, tag="ea2")
                          nc.scalar.activation(out=ea2[:], in_=av, func=AF.Exp,
                                               scale=NEG_SLOPE)
                          mask_all = mp2.tile([P, nci, P], BF16, tag="mk2")
                          nc.vector.tensor_tensor(
                              out=mask_all[:],
                              in0=sl2s[:, ch0:ch0 + nci].rearrange(
                                  "p (c o) -> p c o", o=1).to_broadcast([P, nci, P]),
                              in1=iot[:].rearrange("p (o f) -> p o f", o=1
                                                   ).to_broadcast([P, nci, P]),
                              op=OP.is_equal)
                          w1a = mp2.tile([P, nci, P], BF16, tag="w1a")
                          w2a = mp2.tile([P, nci, P], BF16, tag="w2a")
                          j0 = 0
                          while j0 < nci:
                              b0 = L2["chunks"][ch0 + j0][2]
                              j1 = j0
                              while j1 < nci and L2["chunks"][ch0 + j1][2] == b0:
                                  j1 += 1
                              bi0 = b0 - sg * SGB
                              if bi0 not in adt:
                                  ast = ap2.tile([1, P], F32, tag=f"ast{bi0}")
                                  nc.sync.dma_start(out=ast[:],
                                                    in_=adsto[b0:b0 + 1, :])
                                  at = ap2.tile([P, P], F32, tag=f"adt{bi0}")
                                  nc.gpsimd.partition_broadcast(at[:], ast[:])
                                  e1b = ap2.tile([P, P], F32, tag=f"e1b{bi0}")
                                  nc.scalar.activation(out=e1b[:], in_=at[:], func=AF.Exp)
                                  e2b = ap2.tile([P, P], F32, tag=f"e2b{bi0}")
                                  nc.scalar.activation(out=e2b[:], in_=at[:], func=AF.Exp,
                                                       scale=NEG_SLOPE)
                                  adt[bi0] = (e1b, e2b)
                              e1b, e2b = adt[bi0]
                              nb = j1 - j0
                              nc.vector.tensor_tensor(
                                  out=w1a[:, j0:j1, :],
                                  in0=e1b[:].rearrange("p (o f) -> p o f", o=1
                                                       ).to_broadcast([P, nb, P]),
                                  in1=ea1[:, j0:j1].rearrange(
                                      "p (c o) -> p c o", o=1).to_broadcast([P, nb, P]),
                                  op=OP.mult)
                              nc.vector.tensor_tensor(
                                  out=w2a[:, j0:j1, :],
                                  in0=e2b[:].rearrange("p (o f) -> p o f", o=1
                                                       ).to_broadcast([P, nb, P]),
                                  in1=ea2[:, j0:j1].rearrange(
                                      "p (c o) -> p c o", o=1).to_broadcast([P, nb, P]),
                                  op=OP.mult)
                              j0 = j1
                          wm = mp2.tile([P, nci, P], BF16, tag="wm")
                          nc.vector.tensor_max(out=wm[:], in0=w1a[:], in1=w2a[:])
                          scat_all = mp2.tile([P, nci, P], BF16, tag="sc2")
                          nc.vector.tensor_tensor(out=scat_all[:], in0=wm[:],
                                                  in1=mask_all[:], op=OP.mult)
                          for j in range(nci):
                              cg = ch0 + j
                              _, _, b, first, last = L2["chunks"][cg]
                              bi = b - sg * SGB
                              scat = scat_all[:, j, :]
                              lst = cg == sg_last2[sg]
                              nc.tensor.matmul(out=Qa[:, bi * P:(bi + 1) * P],
                                               lhsT=gt[:, j, 0:P], rhs=scat,
                                               start=False, stop=lst)
                              nc.tensor.matmul(out=Qb[:, bi * P:(bi + 1) * P],
                                               lhsT=gt[:, j, P:2 * P], rhs=scat,
                                               start=False, stop=lst)
                              nc.tensor.matmul(out=den[:, bi:bi + 1], lhsT=scat,
                                               rhs=ones_bf[:], start=False, stop=lst)
                      Aa = ep2.tile([P, SGB * P], F32, tag="Aa")
                      nc.vector.tensor_copy(out=Aa[:, :nblk * P], in_=Qa[:, :nblk * P])
                      Ab = ep2.tile([P, SGB * P], F32, tag="Ab")
                      nc.vector.tensor_copy(out=Ab[:, :nblk * P], in_=Qb[:, :nblk * P])
                      for bi in range(nblk):
                          b = sg * SGB + bi
                          o2 = pp2.tile([P, F3], F32, tag="O2")
                          nc.tensor.matmul(out=o2[:], lhsT=Aa[:, bi * P:(bi + 1) * P],
                                           rhs=w2as[:], start=True, stop=False)
                          nc.tensor.matmul(out=o2[:], lhsT=Ab[:, bi * P:(bi + 1) * P],
                                           rhs=w2bs[:], start=False, stop=True)
                          dmx = ep2.tile([P, 1], F32, tag="dmx")
                          nc.vector.tensor_scalar(out=dmx[:], in0=den[:, bi:bi + 1],
                                                  scalar1=1e-30, scalar2=None, op0=OP.max)
                          rec = ep2.tile([P, 1], F32, tag="rec")
                          nc.vector.reciprocal(out=rec[:], in_=dmx[:])
                          oo = ep2.tile([P, F3], F32, tag="oo")
                          nc.vector.scalar_tensor_tensor(
                              out=oo[:], in0=o2[:], scalar=rec[:], in1=b2s[:],
                              op0=OP.mult, op1=OP.add)
                          nc.sync.dma_start(out=outy[b * P:(b + 1) * P, :], in_=oo[:])
    nc.compile()
    return nc


_CACHE = {}


def kernel(**inputs):
    x = inputs["x"]
    ei = inputs["edge_index"]
    n, f1 = x.shape
    f2 = inputs["W1"].shape[1]
    f3 = inputs["W2"].shape[1]
    cfg = Cfg(n, ei.shape[1], f1, f2, f3)
    meta, in_maps = preprocess(cfg, inputs)
    key = (n, ei.shape[1], f1, f2, f3,
           tuple(i[:3] + (i[2],) for i in meta["L1"]["insts"][:0]))  # cfg key
    skey = (n, f1, f2, f3, meta["L1"]["C"], meta["L2"]["C"],
            meta["L1"]["cols"], meta["L2"]["cols"],
            tuple(c[2] for c in meta["L1"]["chunks"]),
            tuple(c[2] for c in meta["L2"]["chunks"]))
    if skey not in _CACHE:
        _CACHE[skey] = build(cfg, meta)
    ncmod = _CACHE[skey]
    kernel._last = (ncmod, in_maps, cfg)
    res = run_bass_kernel_spmd(ncmod, in_maps, core_ids=list(range(NC)))
    out = np.concatenate(
        [res.results[k]["outy"][:cfg.NLOC] for k in range(NC)], axis=0)
    return out.astype(np.float32)

